# revision 1
# baseline (speedup 1.0000x reference)
"""Trainium2 Bass kernel: channel-attention MultiHeadAttention block.

Full (unsharded) inputs in, full output out. Internally: data-parallel over
batch B across 8 NeuronCores (1 batch each), with a tiny AllReduce for the
BatchNorm batch statistics.

Per-core math (batch b), all shapes [partition, free]:
  qsb/ksb/vsb   [65, 4096]   inputs + ones row (bias fold)
  wqe/wke/wve   [65, 512]    [W.T; bias]
  QT_m, KT_m    [128, 512]   projections transposed (m on partitions)
  scores        [128c, 512d] = sum_m QT[:,c-chunk].T @ KT      (4 psum tiles)
  attn          exp(scores/64) via ACT (+row sums)
  attnT         PE-transpose blocks fused with diag(1/rowsum)
  V             [128d, 4096m] natural layout
  X[cc]         [128c', 4096s'] = attn@V directly in post-permute BN layout,
                via stride-8 m-slices of V as the matmul stationary operand
  BN stats      row sums/sumsq -> AllReduce over 8 cores -> alpha/beta
  BN+leaky      in-place ACT Lrelu(scale=alpha, bias=beta)
  w1 + leaky    [512,512] conv, bias+leaky fused in ACT
  w2 + bias     [64,512] conv -> y [64, 4096]
"""

import sys

if "/opt/trn_rl_repo" not in sys.path:
    sys.path.insert(0, "/opt/trn_rl_repo")

import numpy as np

import concourse.bacc as bacc
import concourse.mybir as mybir
import concourse.tile as tile
from concourse import bass_utils

B = 8
C = 64
CN = 512
HW = 4096
NM = HW // 128   # 32 m-chunks
NCH = CN // 128  # 4 channel chunks
NS = HW // 512   # 8 free-dim slices
EPS = 1e-4
SLOPE = 0.01
INV_SCALE = 1.0 / 64.0      # 1/sqrt(HW)
INV_BHW = 1.0 / (B * HW)    # BN divisor

F32 = mybir.dt.float32
F32R = mybir.dt.float32r
AF = mybir.ActivationFunctionType
ALU = mybir.AluOpType
AX = mybir.AxisListType
RG = [[0, 1, 2, 3, 4, 5, 6, 7]]


def _r(ap):
    return ap.bitcast(F32R)


def _body(tc, nc, d, dbg=None):
    with (
        tc.tile_pool(name="consts", bufs=1) as consts,
        tc.tile_pool(name="small", bufs=1) as small,
        tc.tile_pool(name="atp", bufs=1) as atp,
        tc.tile_pool(name="vbuf", bufs=1) as vpool,
    ):
        # ---- weights / constants: packed tile, split DMA (proj part 1st)
        wpack = consts.tile([128, 4000], F32R, name="wpack", tag="wpack")
        nc.sync.dma_start(wpack[:, 0:1536], d["wpack"][:, 0:1536])
        nc.scalar.dma_start(wpack[:, 1536:4000], d["wpack"][:, 1536:4000])
        wqe = wpack[0:65, 0:512]
        wke = wpack[0:65, 512:1024]
        wve = wpack[0:65, 1024:1536]
        w1sb = [wpack[:, 1536 + 512 * cc:1536 + 512 * (cc + 1)]
                for cc in range(NCH)]
        w2sb = [wpack[:, 3584 + 64 * oc:3584 + 64 * (oc + 1)]
                for oc in range(NCH)]
        ident = wpack[:, 3840:3968]
        b1sb = wpack[:, 3968:3972].bitcast(F32)
        b2sb = wpack[0:64, 3972:3973].bitcast(F32)
        bngsb = wpack[:, 3976:3980].bitcast(F32)
        bnbsb = wpack[:, 3980:3984].bitcast(F32)

        # warmup collective: absorbs ncfw cold-start + inter-core skew early
        with tc.tile_pool(name="wudram", bufs=1, space="DRAM") as wud:
            dwin = wud.tile([128, 1], F32, name="dwin", tag="dwin")
            dwout = wud.tile([128, 1], F32, name="dwout", tag="dwout")
            wsrc = small.tile([128, 1], F32, name="wsrc", tag="wsrc")
            nc.gpsimd.memset(wsrc[:], 1.0)
            nc.gpsimd.dma_start(dwin[:], wsrc[:])
            nc.gpsimd.collective_compute(
                "AllReduce", ALU.add, replica_groups=RG,
                ins=[dwin.opt()], outs=[dwout.opt()])
            wdst = small.tile([128, 1], F32, name="wdst", tag="wdst")
            nc.gpsimd.dma_start(wdst[:], dwout[:])

        alpha = small.tile([128, 4], F32, name="alpha", tag="alpha")
        beta = small.tile([128, 4], F32, name="beta", tag="beta")
        epsb = small.tile([128, 1], F32, name="epsb", tag="epsb")
        nc.gpsimd.memset(epsb[:], EPS)

        V = [vpool.tile([128, HW], F32R, name=f"V{dc}", tag=f"V{dc}")
             for dc in range(NCH)]
        aT = [atp.tile([128, CN], F32R, name=f"aT{dc}", tag=f"aT{dc}")
              for dc in range(NCH)]

        # ================= phase 1: proj + scores + softmax + V ==========
        with (
            tc.tile_pool(name="inp", bufs=1) as inp,
            tc.tile_pool(name="qk", bufs=2) as qkp,
            tc.tile_pool(name="attn", bufs=1) as ap_,
        ):
            qsb = inp.tile([65, HW], F32R, name="qsb", tag="qsb")
            ksb = inp.tile([65, HW], F32R, name="ksb", tag="ksb")
            vsb = inp.tile([65, HW], F32R, name="vsb", tag="vsb")
            nc.sync.dma_start(qsb[:, 0:2048], d["q"][:, 0:2048])
            nc.gpsimd.dma_start(ksb[:, 0:2048], d["k"][:, 0:2048])
            nc.sync.dma_start(qsb[:, 2048:4096], d["q"][:, 2048:4096])
            nc.gpsimd.dma_start(ksb[:, 2048:4096], d["k"][:, 2048:4096])
            nc.scalar.dma_start(vsb[:, 0:2048], d["v"][:, 0:2048])
            nc.scalar.dma_start(vsb[:, 2048:4096], d["v"][:, 2048:4096])

            rowsum = ap_.tile([128, 4], F32, name="rowsum", tag="rowsum")
            recip = ap_.tile([128, 4], F32, name="recip", tag="recip")
            attn = [ap_.tile([128, 512], F32R, name=f"attn{cc}",
                             tag=f"attn{cc}") for cc in range(NCH)]

            with (
                tc.tile_pool(name="scps", bufs=1, space="PSUM") as scps,
                tc.tile_pool(name="pjps", bufs=1, space="PSUM") as pjps,
            ):
                sc = [scps.tile([128, 512], F32, name=f"sc{cc}",
                                tag=f"sc{cc}") for cc in range(NCH)]
                # 2 m-chunks per iteration, software-pipelined one
                # iteration ahead so PE never waits on the psum->sbuf copies
                NIT = NM // 2

                def proj(it):
                    qtp = pjps.tile([128, 1024], F32, name="qtp", tag="qtp")
                    ktp = pjps.tile([128, 1024], F32, name="ktp", tag="ktp")
                    for h in range(2):
                        msl = slice(128 * (2 * it + h),
                                    128 * (2 * it + h + 1))
                        nc.tensor.matmul(qtp[:, 512 * h:512 * (h + 1)],
                                         qsb[:, msl], wqe,
                                         start=True, stop=True)
                        nc.tensor.matmul(ktp[:, 512 * h:512 * (h + 1)],
                                         ksb[:, msl], wke,
                                         start=True, stop=True)
                    return qtp, ktp

                def copies(qtp, ktp):
                    qts = qkp.tile([128, 1024], F32R, name="qts", tag="qts")
                    nc.scalar.copy(qts[:], qtp[:])
                    kts = qkp.tile([128, 1024], F32R, name="kts", tag="kts")
                    nc.vector.tensor_copy(kts[:], ktp[:])
                    return qts, kts

                def score_mms(it, qts, kts):
                    for h in range(2):
                        for cc in range(NCH):
                            nc.tensor.matmul(
                                sc[cc][:],
                                qts[:, 512 * h + 128 * cc:
                                    512 * h + 128 * (cc + 1)],
                                kts[:, 512 * h:512 * (h + 1)],
                                start=(it == 0 and h == 0),
                                stop=(it == NIT - 1 and h == 1))

                pq = proj(0)
                cur = copies(*pq)
                nxt_p = proj(1)
                for it in range(NIT):
                    score_mms(it, *cur)
                    if it + 1 < NIT:
                        cur = copies(*nxt_p)
                    if it + 2 < NIT:
                        nxt_p = proj(it + 2)

                # softmax exp on ACT (overlaps V projection below on PE)
                for cc in range(NCH):
                    nc.scalar.activation(attn[cc][:], sc[cc][:], AF.Exp,
                                         bias=0.0, scale=INV_SCALE,
                                         accum_out=rowsum[:, cc:cc + 1])

                # V projection (natural [d, m] layout)
                for dc in range(NCH):
                    for mp in range(NS // 2):
                        vtp = pjps.tile([128, 1024], F32, name="vtp",
                                        tag=("qtp" if mp % 2 == 0 else "ktp"))
                        for h in range(2):
                            ssl = slice(1024 * mp + 512 * h,
                                        1024 * mp + 512 * (h + 1))
                            nc.tensor.matmul(
                                vtp[:, 512 * h:512 * (h + 1)],
                                wve[:, 128 * dc:128 * (dc + 1)],
                                vsb[:, ssl], start=True, stop=True)
                        vsl = slice(1024 * mp, 1024 * (mp + 1))
                        if (dc + mp) % 2 == 0:
                            nc.scalar.copy(V[dc][:, vsl], vtp[:])
                        else:
                            nc.vector.tensor_copy(V[dc][:, vsl], vtp[:])

                # normalize attn rows (DVE)
                for cc in range(NCH):
                    nc.vector.reciprocal(recip[:, cc:cc + 1],
                                         rowsum[:, cc:cc + 1])
                    nc.vector.tensor_scalar_mul(attn[cc][:], attn[cc][:],
                                                recip[:, cc:cc + 1])

            # transposes: 4 blocks of one dc into one 512-wide psum tile
            with tc.tile_pool(name="tps", bufs=2, space="PSUM") as tps:
                for dc in range(NCH):
                    tp = tps.tile([128, 512], F32, name="tp", tag="tp")
                    for cc in range(NCH):
                        nc.tensor.transpose(
                            _r(tp[:, 128 * cc:128 * (cc + 1)]),
                            attn[cc][:, 128 * dc:128 * (dc + 1)],
                            ident)
                    nc.scalar.copy(aT[dc][:], tp[:])

            if dbg is not None:
                for cc in range(NCH):
                    nc.sync.dma_start(dbg[f"attn{cc}"][:], attn[cc][:])

        if dbg is not None:
            for dc in range(NCH):
                nc.sync.dma_start(dbg[f"aT{dc}"][:], aT[dc][:])
                nc.sync.dma_start(dbg[f"V{dc}"][:], V[dc][:])

        # ============ phase 2: attn@V -> X (BN layout) + stats + AR ======
        with (
            tc.tile_pool(name="xbuf", bufs=1) as xpool,
            tc.tile_pool(name="stp", bufs=2) as stp,
            tc.tile_pool(name="scr", bufs=2) as scr,
            tc.tile_pool(name="cdram", bufs=1, space="DRAM") as cdram,
        ):
            X = [xpool.tile([128, HW], F32R, name=f"X{cc}", tag=f"X{cc}")
                 for cc in range(NCH)]
            red = stp.tile([128, 8], F32, name="red", tag="red", bufs=1)
            with tc.tile_pool(name="xps", bufs=3, space="PSUM") as xps:
                for cc in range(NCH):
                    Vr = [V[dc].rearrange("d (cc t lo) -> d cc lo t",
                                          cc=4, lo=8) for dc in range(NCH)]
                    ps_sum = stp.tile([128, 4], F32, name="pssum", tag="pssum")
                    ps_sq = stp.tile([128, 4], F32, name="pssq", tag="pssq")
                    for lp in range(4):
                        xt = xps.tile([128, 1024], F32, name="xt", tag="xt")
                        for h in range(2):
                            lo = 2 * lp + h
                            for dc in range(NCH):
                                nc.tensor.matmul(
                                    xt[:, 512 * h:512 * (h + 1)],
                                    Vr[dc][:, cc, lo, :], aT[dc][:],
                                    start=(dc == 0), stop=(dc == 3))
                        xsl = slice(1024 * lp, 1024 * (lp + 1))
                        nc.vector.tensor_scalar(
                            out=X[cc][:, xsl], in0=xt[:], scalar1=1.0,
                            scalar2=0.0, op0=ALU.mult, op1=ALU.add,
                            accum_out=ps_sum[:, lp:lp + 1])
                        junk = scr.tile([128, 1024], F32, name="junk",
                                        tag="junk")
                        nc.scalar.activation(junk[:], X[cc][:, xsl],
                                             AF.Square,
                                             accum_out=ps_sq[:, lp:lp + 1])
                    nc.vector.reduce_sum(red[:, 2 * cc:2 * cc + 1],
                                         ps_sum[:], axis=AX.X)
                    nc.vector.reduce_sum(red[:, 2 * cc + 1:2 * cc + 2],
                                         ps_sq[:], axis=AX.X)

                # single AllReduce for all BN stats
                cin = cdram.tile([128, 8], F32, name="cin", tag="cin")
                cout = cdram.tile([128, 8], F32, name="cout", tag="cout")
                nc.sync.dma_start(cin[:], red[:])
                nc.gpsimd.collective_compute(
                    "AllReduce", ALU.add, replica_groups=RG,
                    ins=[cin.opt()], outs=[cout.opt()])
                ar = stp.tile([128, 8], F32, name="ar", tag="ar", bufs=1)
                nc.sync.dma_start(ar[:], cout[:])
                # preload ACT tables while the AllReduce is in flight
                dummy = stp.tile([128, 1], F32, name="dummy", tag="dummy",
                                 bufs=1)
                nc.scalar.activation(dummy[:], epsb[:], AF.Lrelu,
                                     bias=0.0, scale=1.0, alpha=SLOPE)
                nc.scalar.activation(dummy[:], epsb[:], AF.Sqrt,
                                     bias=epsb[:, 0:1])

                # BN affine params (tiny DVE/ACT ops)
                mean = stp.tile([128, 4], F32, name="mean", tag="mean",
                                bufs=1)
                var = stp.tile([128, 4], F32, name="var", tag="var", bufs=1)
                sd = stp.tile([128, 4], F32, name="sd", tag="sd", bufs=1)
                rstd = stp.tile([128, 4], F32, name="rstd", tag="rstd",
                                bufs=1)
                tmp = stp.tile([128, 4], F32, name="tmpb", tag="tmpb",
                               bufs=1)
                for cc in range(NCH):
                    nc.vector.tensor_scalar_mul(mean[:, cc:cc + 1],
                                                ar[:, 2 * cc:2 * cc + 1],
                                                INV_BHW)
                    nc.vector.tensor_scalar_mul(var[:, cc:cc + 1],
                                                ar[:, 2 * cc + 1:2 * cc + 2],
                                                INV_BHW)
                nc.vector.tensor_mul(tmp[:], mean[:], mean[:])
                nc.vector.tensor_sub(var[:], var[:], tmp[:])
                nc.scalar.activation(sd[:], var[:], AF.Sqrt,
                                     bias=epsb[:, 0:1])
                nc.vector.reciprocal(rstd[:], sd[:])
                nc.vector.tensor_mul(alpha[:], bngsb[:], rstd[:])
                nc.vector.tensor_mul(tmp[:], mean[:], alpha[:])
                nc.vector.tensor_sub(beta[:], bnbsb[:], tmp[:])

                if dbg is not None:
                    for cc in range(NCH):
                        nc.sync.dma_start(dbg[f"X{cc}"][:], X[cc][:])
                        nc.sync.dma_start(dbg[f"ar{cc}"][:, 0:1],
                                          ar[:, 2 * cc:2 * cc + 1])
                        nc.sync.dma_start(dbg[f"ar{cc}"][:, 1:2],
                                          ar[:, 2 * cc + 1:2 * cc + 2])
                        nc.sync.dma_start(dbg[f"ab{cc}"][:, 0:1],
                                          alpha[:, cc:cc + 1])
                        nc.sync.dma_start(dbg[f"ab{cc}"][:, 1:2],
                                          beta[:, cc:cc + 1])

                # BN + leaky, in place; first 512 cols of each chunk
                # first so the w1 phase unblocks early
                for xsl in (slice(0, 512), slice(512, 2048),
                            slice(2048, 4096)):
                    for cc in range(NCH):
                        nc.scalar.activation(X[cc][:, xsl], X[cc][:, xsl],
                                             AF.Lrelu,
                                             bias=beta[:, cc:cc + 1],
                                             scale=alpha[:, cc:cc + 1],
                                             alpha=SLOPE)

            # ================= phase 3: w1 -> leaky -> w2 -> y ===========
            with (
                tc.tile_pool(name="y2", bufs=2) as y2p,
                tc.tile_pool(name="outb", bufs=1) as outp,
                tc.tile_pool(name="wps", bufs=1, space="PSUM") as wps,
                tc.tile_pool(name="w2ps", bufs=2, space="PSUM") as w2ps,
            ):
                osb = outp.tile([64, HW], F32, name="osb", tag="osb")
                for ms in range(NS):
                    ssl = slice(512 * ms, 512 * (ms + 1))
                    y2t = []
                    for oc in range(NCH):
                        wp = wps.tile([128, 512], F32, name=f"wp{oc}",
                                      tag=f"wp{oc}")
                        for cc in range(NCH):
                            nc.tensor.matmul(
                                wp[:],
                                w1sb[cc][:, 128 * oc:128 * (oc + 1)],
                                X[cc][:, ssl],
                                start=(cc == 0), stop=(cc == 3))
                        yt = y2p.tile([128, 512], F32R, name=f"y2_{oc}",
                                      tag=f"y2_{oc}")
                        nc.scalar.activation(yt[:], wp[:], AF.Lrelu,
                                             bias=b1sb[:, oc:oc + 1],
                                             scale=1.0, alpha=SLOPE)
                        y2t.append(yt)
                    fp = w2ps.tile([64, 512], F32, name="fp", tag="fp")
                    for oc in range(NCH):
                        nc.tensor.matmul(fp[:], w2sb[oc], y2t[oc][:],
                                         start=(oc == 0), stop=(oc == 3))
                    nc.vector.tensor_scalar_add(osb[:, ssl], fp[:],
                                                b2sb[:, 0:1])
                    nc.sync.dma_start(d["y"][:, ssl], osb[:, ssl])


_NC_CACHE = {}


def _build(debug=False):
    key = ("dbg" if debug else "nc")
    if key in _NC_CACHE:
        return _NC_CACHE[key]
    nc = bacc.Bacc(trn_type="TRN2", target_bir_lowering=False, debug=False,
                   enable_asserts=False, num_devices=8)
    d = {}
    d["q"] = nc.dram_tensor("q", (65, HW), F32R, kind="ExternalInput").ap()
    d["k"] = nc.dram_tensor("k", (65, HW), F32R, kind="ExternalInput").ap()
    d["v"] = nc.dram_tensor("v", (65, HW), F32R, kind="ExternalInput").ap()
    d["wpack"] = nc.dram_tensor("wpack", (128, 4000), F32R,
                                kind="ExternalInput").ap()
    d["y"] = nc.dram_tensor("y", (64, HW), F32, kind="ExternalOutput").ap()

    dbg = None
    if debug:
        dbg = {}
        for cc in range(NCH):
            dbg[f"attn{cc}"] = nc.dram_tensor(f"dbg_attn{cc}", (128, 512), F32R, kind="ExternalOutput").ap()
            dbg[f"aT{cc}"] = nc.dram_tensor(f"dbg_aT{cc}", (128, 512), F32R, kind="ExternalOutput").ap()
            dbg[f"V{cc}"] = nc.dram_tensor(f"dbg_V{cc}", (128, HW), F32R, kind="ExternalOutput").ap()
            dbg[f"X{cc}"] = nc.dram_tensor(f"dbg_X{cc}", (128, HW), F32R, kind="ExternalOutput").ap()
            dbg[f"ar{cc}"] = nc.dram_tensor(f"dbg_ar{cc}", (128, 2), F32, kind="ExternalOutput").ap()
            dbg[f"ab{cc}"] = nc.dram_tensor(f"dbg_ab{cc}", (128, 2), F32, kind="ExternalOutput").ap()
    with tile.TileContext(nc) as tc:
        _body(tc, nc, d, dbg)
    nc.compile()
    _NC_CACHE[key] = nc
    return nc


def _run(q, k, v, wq, bq, wk, bk, wv, bv, bn_g, bn_b, w1, b1, w2, b2,
         trace=False, tmpdir=None, debug=False):
    nc = _build(debug)
    f = np.float32
    wpack = np.zeros((128, 4000), f)
    wpack[0:65, 0:512] = np.concatenate([wq.T, bq[None, :]], axis=0)
    wpack[0:65, 512:1024] = np.concatenate([wk.T, bk[None, :]], axis=0)
    wpack[0:65, 1024:1536] = np.concatenate([wv.T, bv[None, :]], axis=0)
    w1t = w1.T.astype(f)
    for cc in range(4):
        wpack[:, 1536 + 512 * cc:1536 + 512 * (cc + 1)] = \
            w1t[128 * cc:128 * (cc + 1), :]
    w2t = w2.T.astype(f)
    for oc in range(4):
        wpack[:, 3584 + 64 * oc:3584 + 64 * (oc + 1)] = \
            w2t[128 * oc:128 * (oc + 1), :]
    wpack[:, 3840:3968] = np.eye(128, dtype=f)
    wpack[:, 3968:3972] = b1.reshape(4, 128).T
    wpack[0:64, 3972] = b2
    wpack[:, 3976:3980] = bn_g.reshape(4, 128).T
    wpack[:, 3980:3984] = bn_b.reshape(4, 128).T
    shared = {"wpack": wpack}
    in_maps = []
    for b in range(B):
        m = dict(shared)
        ones = np.ones((1, HW), f)
        m["q"] = np.concatenate([q[b].reshape(64, HW), ones], axis=0).astype(f)
        m["k"] = np.concatenate([k[b].reshape(64, HW), ones], axis=0).astype(f)
        m["v"] = np.concatenate([v[b].reshape(64, HW), ones], axis=0).astype(f)
        in_maps.append(m)
    res = bass_utils.run_bass_kernel_spmd(
        nc, in_maps, core_ids=list(range(8)), trace=trace, tmpdir=tmpdir)
    out = np.stack([res.results[b]["y"].reshape(C, 64, 64) for b in range(B)])
    return out.astype(np.float32), res


def kernel(q, k, v, wq, bq, wk, bk, wv, bv, bn_g, bn_b, w1, b1, w2, b2):
    out, _ = _run(q, k, v, wq, bq, wk, bk, wv, bv, bn_g, bn_b, w1, b1, w2, b2)
    return out



# revision 18
# speedup vs baseline: 1.4946x; 1.4946x over previous
"""Trainium2 Bass kernel: channel-attention MultiHeadAttention block.

Full (unsharded) inputs in, full output out. Data-parallel over batch B
across 8 NeuronCores (1 batch each), tiny AllReduce for BatchNorm stats.

Algorithmic structure (per core, batch b), exploiting the 1x1-conv
low-rank structure of the attention:

  scores = (Wq q)(Wk k)^T = Wq (q k^T) Wk^T   with  G = q k^T  [65,65]
  out    = softmax(scores) (Wv v) = H v        with  H = attn Wv [512,65]

Everything is computed in TRANSPOSED orientation (ST[d,c]) so that the
softmax normalizer and the BatchNorm statistics can be derived from
small factored matrices *before* the big X tensor is materialized:

  sum_j X[a,j]   = sum_t W[t,a] u[t]          W = fold_r(v), u = HTn 1
  sum_j X[a,j]^2 = sum_r diag(v_r^T M v_r)    M = HTn HTn^T  [65,65]

so the BN AllReduce is fired early and overlaps X production.
All heavy matmuls run in bf16 (1 cyc/row, FWL weight loads, no
fp32_mode=HIGH power throttle).
"""

import sys

if "/opt/trn_rl_repo" not in sys.path:
    sys.path.insert(0, "/opt/trn_rl_repo")

import ml_dtypes
import numpy as np

import concourse.bacc as bacc
import concourse.mybir as mybir
import concourse.tile as tile
from concourse import bass_utils

B = 8
C = 64
CN = 512
HW = 4096
NCH = 4         # 128-chunks of the 512 dims
NS = 8          # 512-wide spatial slices
NMC = 32        # 128-wide m-chunks
EPS = 1e-4
SLOPE = 0.01
INV_SCALE = 1.0 / 64.0
INV_BHW = 1.0 / (B * HW)

F32 = mybir.dt.float32
F32R = mybir.dt.float32r
BF16 = mybir.dt.bfloat16
AF = mybir.ActivationFunctionType
ALU = mybir.AluOpType
AX = mybir.AxisListType
RG = [[0, 1, 2, 3, 4, 5, 6, 7]]

# wpack (bf16) column layout
WQE0 = 0
WKE0 = 512
WVT0 = 1024
W1T0 = 1284
W2T0 = W1T0 + 2048          # 3332
IDB0 = W2T0 + 256           # 3588
WCOLS = IDB0 + 128          # 3716


def _r(ap):
    return ap.bitcast(F32R)


def _body(tc, nc, d, dbg=None):
    with (
        tc.tile_pool(name="consts", bufs=1) as consts,
        tc.tile_pool(name="small", bufs=1) as small,
        tc.tile_pool(name="xbuf", bufs=1) as xpool,
    ):
        # ---- weights / constants
        wpack = consts.tile([128, WCOLS], BF16, name="wpack", tag="wpack")
        nc.scalar.dma_start(wpack[:, 0:1284], d["wpack"][:, 0:1284])
        nc.scalar.dma_start(wpack[:, 1284:WCOLS], d["wpack"][:, 1284:WCOLS])
        spack = consts.tile([128, 16], F32, name="spack", tag="spack")
        nc.sync.dma_start(spack[:], d["spack"][:])
        wqe = wpack[0:65, WQE0:WQE0 + 512]
        wke = wpack[0:65, WKE0:WKE0 + 512]
        b1sb = spack[:, 0:4]
        bngsb = spack[:, 4:8]
        bnbsb = spack[:, 8:12]
        b2sb = spack[0:64, 12:13]
        identb = wpack[:, IDB0:IDB0 + 128]

        # ---- inputs
        qt = consts.tile([128, 2080], BF16, name="qt", tag="qt")
        kt = consts.tile([128, 2080], BF16, name="kt", tag="kt")
        vt = consts.tile([65, HW], F32R, name="vt", tag="vt")
        nc.sync.dma_start(qt[:], d["qt"][:])
        nc.sync.dma_start(kt[:], d["kt"][:])
        nc.gpsimd.dma_start(vt[:, 0:2048], d["vt"][:, 0:2048])
        nc.gpsimd.dma_start(vt[:, 2048:4096], d["vt"][:, 2048:4096])

        # warmup collective: absorbs ncfw cold-start + inter-core skew early
        with tc.tile_pool(name="wudram", bufs=1, space="DRAM") as wud:
            dwin = wud.tile([128, 1], F32, name="dwin", tag="dwin")
            dwout = wud.tile([128, 1], F32, name="dwout", tag="dwout")
            wsrc = small.tile([128, 1], F32, name="wsrc", tag="wsrc")
            nc.gpsimd.memset(wsrc[:], 1.0)
            nc.gpsimd.dma_start(dwin[:], wsrc[:])
            nc.gpsimd.collective_compute(
                "AllReduce", ALU.add, replica_groups=RG,
                ins=[dwin.opt()], outs=[dwout.opt()])
            wdst = small.tile([128, 1], F32, name="wdst", tag="wdst")
            nc.gpsimd.dma_start(wdst[:], dwout[:])

        ones65 = small.tile([65, 1], F32, name="ones65", tag="ones65")
        nc.gpsimd.memset(ones65[:], 1.0)
        epsb = small.tile([128, 1], F32, name="epsb", tag="epsb")
        nc.gpsimd.memset(epsb[:], EPS)
        ones128 = small.tile([128, 1], BF16, name="ones128", tag="ones128")
        nc.gpsimd.memset(ones128[:], 1.0)
        onesf = small.tile([1, 1], F32, name="onesf", tag="onesf")
        nc.gpsimd.memset(onesf[:], 1.0)

        X = [xpool.tile([128, HW], BF16, name=f"X{cc}", tag=f"X{cc}")
             for cc in range(NCH)]
        HTn = small.tile([65, 512], F32R, name="HTn", tag="HTn")
        alpha = small.tile([128, 4], F32, name="alpha", tag="alpha")
        beta = small.tile([128, 4], F32, name="beta", tag="beta")

        # W = fold_r(v): W[t,a] = sum_r v[t, 8a+r]  (only needs vt)
        Wf = small.tile([65, 512], F32, name="Wf", tag="Wf")
        nc.vector.reduce_sum(Wf[:], vt.rearrange("t (a r) -> t a r", r=8),
                             axis=AX.X)

        # ============ phase 1: gram + score chain + softmax(T) ==========
        with tc.tile_pool(name="ph1", bufs=1) as ph1:
            with tc.tile_pool(name="gps", bufs=1, space="PSUM") as gps:
                # G[t,j] = sum_m qT[m,t] kT[m,j], accumulated over m-chunks
                g_ps = gps.tile([65, 65], F32, name="g_ps", tag="g_ps")
                for i in range(NMC):
                    nc.tensor.matmul(g_ps[:], qt[:, 65 * i:65 * (i + 1)],
                                     kt[:, 65 * i:65 * (i + 1)],
                                     start=(i == 0), stop=(i == NMC - 1))
                g_sb = ph1.tile([65, 65], BF16, name="g_sb", tag="g_sb")
                nc.vector.tensor_copy(g_sb[:], g_ps[:])

                # Pp = G^T Wq~^T : Pp[j,c] = sum_t G[t,j] wqe[t,c]
                pp_ps = gps.tile([65, 512], F32, name="pp_ps", tag="pp_ps")
                nc.tensor.matmul(pp_ps[:], g_sb[:], wqe, start=True,
                                 stop=True)
                pp_sb = ph1.tile([65, 512], BF16, name="pp_sb", tag="pp_sb")
                nc.scalar.copy(pp_sb[:], pp_ps[:])

            ET = [ph1.tile([128, 512], BF16, name=f"ET{dc}", tag=f"ET{dc}")
                  for dc in range(NCH)]
            recip1 = ph1.tile([1, 512], F32, name="recip1", tag="recip1")
            with tc.tile_pool(name="stps", bufs=1, space="PSUM") as stps:
                # ST[d,c] = sum_j wke[j,d] Pp[j,c]  (4 d-chunks)
                st_ps = [stps.tile([128, 512], F32, name=f"st{dc}",
                                   tag=f"st{dc}") for dc in range(NCH)]
                rs_ps = stps.tile([1, 512], F32, name="rs_ps", tag="rs_ps")
                for dc in range(NCH):
                    nc.tensor.matmul(st_ps[dc][:],
                                     wke[:, 128 * dc:128 * (dc + 1)],
                                     pp_sb[:], start=True, stop=True)
                    nc.scalar.activation(ET[dc][:], st_ps[dc][:], AF.Exp,
                                         bias=0.0, scale=INV_SCALE)
                # colsums of ET via ones-matmul -> [1, 512]
                for dc in range(NCH):
                    nc.tensor.matmul(rs_ps[:], ones128[:], ET[dc][:],
                                     start=(dc == 0), stop=(dc == NCH - 1))
                nc.vector.reciprocal(recip1[:], rs_ps[:])

            with tc.tile_pool(name="hps", bufs=1, space="PSUM") as hps:
                # transpose recip into per-partition orientation [128, 4]
                rp_ps = hps.tile([128, 4], F32, name="rp_ps", tag="rp_ps")
                for j in range(NCH):
                    nc.tensor.transpose(
                        rp_ps[:, j:j + 1],
                        recip1[0:1, 128 * j:128 * (j + 1)],
                        onesf[0:1, 0:1])
                recip_p = small.tile([128, 4], F32, name="recip_p",
                                     tag="recip_p")
                nc.vector.tensor_copy(recip_p[:], rp_ps[:])

                # HTu = Wv~^T E^T : 4 accumulating matmuls
                htu_ps = hps.tile([65, 512], F32, name="htu_ps",
                                  tag="htu_ps")
                for dc in range(NCH):
                    nc.tensor.matmul(
                        htu_ps[:],
                        wpack[:, WVT0 + 65 * dc:WVT0 + 65 * (dc + 1)],
                        ET[dc][:], start=(dc == 0), stop=(dc == NCH - 1))
                htu_sb = ph1.tile([65, 512], BF16, name="htu_sb",
                                  tag="htu_sb")
                nc.scalar.copy(htu_sb[:], htu_ps[:])

                # H_n1[c, t] = HTu^T * recip (transpose + per-part scale)
                # (66-col stride: PSUM writes must be 4B aligned)
                tp_ps = hps.tile([128, 264], BF16, name="tp_ps", tag="tp_ps")
                h_n1 = ph1.tile([128, 260], BF16, name="h_n1", tag="h_n1")
                for j in range(NCH):
                    nc.tensor.transpose(tp_ps[:, 66 * j:66 * j + 65],
                                        htu_sb[:, 128 * j:128 * (j + 1)],
                                        identb[0:65, 0:65])
                    nc.vector.tensor_scalar_mul(
                        h_n1[:, 65 * j:65 * (j + 1)],
                        tp_ps[:, 66 * j:66 * j + 65],
                        recip_p[:, j:j + 1])
                # HTn = transpose back: [65, 512]
                btp_ps = hps.tile([65, 512], BF16, name="btp_ps",
                                  tag="btp_ps")
                for j in range(NCH):
                    nc.tensor.transpose(btp_ps[:, 128 * j:128 * (j + 1)],
                                        h_n1[:, 65 * j:65 * (j + 1)],
                                        identb[:, 0:128])
                nc.scalar.copy(HTn[:], btp_ps[:])

                # M[t,t'] = sum_c Hn1[c,t] Hn1[c,t']
                # (full-bank tile: accumulation groups must own their bank —
                # any start=True matmul clears has_written for the whole bank)
                m_ps = hps.tile([65, 512], F32, name="m_ps", tag="m_ps")
                for j in range(NCH):
                    nc.tensor.matmul(m_ps[:, 0:65],
                                     h_n1[:, 65 * j:65 * (j + 1)],
                                     h_n1[:, 65 * j:65 * (j + 1)],
                                     start=(j == 0), stop=(j == NCH - 1))
                m_sb = small.tile([65, 65], F32R, name="m_sb", tag="m_sb")
                nc.vector.tensor_copy(m_sb[:], m_ps[:, 0:65])
            # u = HTn @ 1 : row sums [65, 1]
            u_sb = small.tile([65, 1], F32, name="u_sb", tag="u_sb")
            nc.vector.reduce_sum(u_sb[:], HTn[:], axis=AX.X)

            if dbg is not None:
                nc.sync.dma_start(dbg["g"][:], g_sb[:])
                for dc in range(NCH):
                    nc.sync.dma_start(dbg[f"ET{dc}"][:], ET[dc][:])
                nc.sync.dma_start(dbg["HTn"][:], HTn[:])
                nc.sync.dma_start(dbg["m"][:], m_sb[:])

        # ============ phase 2: factored BN stats -> AllReduce ===========
        # (emitted before X so the PE prioritizes unblocking the AR)
        vtr = vt.rearrange("t (a r) -> t r a", r=8)
        with (
            tc.tile_pool(name="ph2", bufs=1) as ph2,
            tc.tile_pool(name="ssps", bufs=1, space="PSUM") as ssps,
            tc.tile_pool(name="zps", bufs=2, space="PSUM") as zps,
            tc.tile_pool(name="cdram", bufs=1, space="DRAM") as cdram,
            tc.tile_pool(name="xps", bufs=4, space="PSUM") as xps,
            tc.tile_pool(name="stp", bufs=1) as stp,
        ):
            # stat matmuls are all single-shot writes to distinct columns of
            # one bank (no PSUM accumulation groups — a start=True clears
            # has_written bank-wide); the r-reduction happens on the DVE.
            ss_ps = ssps.tile([128, 36], F32, name="ss_ps", tag="ss_ps")
            # sum[a] = sum_t W[t,a] u[t]
            for cc in range(NCH):
                nc.tensor.matmul(ss_ps[:, cc:cc + 1],
                                 Wf[:, 128 * cc:128 * (cc + 1)],
                                 u_sb[:], start=True, stop=True)
            # P[t, 512r + a] = v_r[t,a] * (M v_r)[t,a]
            P = ph2.tile([65, HW], F32, name="P", tag="P")
            for r in range(NS):
                z_ps = zps.tile([65, 512], F32, name="z_ps", tag="z_ps")
                nc.tensor.matmul(z_ps[:], m_sb[:], vtr[:, r, :],
                                 start=True, stop=True)
                nc.vector.tensor_mul(P[:, 512 * r:512 * (r + 1)],
                                     vtr[:, r, :], z_ps[:])
            # sumsq part (cc, r) -> column 4 + 8*cc + r
            for r in range(NS):
                for cc in range(NCH):
                    nc.tensor.matmul(
                        ss_ps[:, 4 + 8 * cc + r:5 + 8 * cc + r],
                        P[:, 512 * r + 128 * cc:512 * r + 128 * (cc + 1)],
                        ones65[:], start=True, stop=True)
            red = stp.tile([128, 8], F32, name="red", tag="red")
            nc.vector.tensor_copy(red[:, 0:4], ss_ps[:, 0:4])
            nc.vector.reduce_sum(
                red[:, 4:8],
                ss_ps[:, 4:36].rearrange("p (cc r) -> p cc r", r=8),
                axis=AX.X)

            cin = cdram.tile([128, 8], F32, name="cin", tag="cin")
            cout = cdram.tile([128, 8], F32, name="cout", tag="cout")
            nc.sync.dma_start(cin[:], red[:])
            nc.gpsimd.collective_compute(
                "AllReduce", ALU.add, replica_groups=RG,
                ins=[cin.opt()], outs=[cout.opt()])
            ar = stp.tile([128, 8], F32, name="ar", tag="ar")
            nc.sync.dma_start(ar[:], cout[:])

            # ======== phase 3: X = v_r^T HTn (overlaps the AllReduce) ====
            for cc in range(NCH):
                for r in range(NS):
                    xt = xps.tile([128, 512], F32, name="xt", tag="xt")
                    nc.tensor.matmul(xt[:], vtr[:, r, 128 * cc:128 * (cc + 1)],
                                     HTn[:], start=True, stop=True)
                    xsl = slice(512 * r, 512 * (r + 1))
                    if r % 2 == 0:
                        nc.scalar.copy(X[cc][:, xsl], xt[:])
                    else:
                        nc.vector.tensor_copy(X[cc][:, xsl], xt[:])

            # preload ACT tables while the AllReduce is in flight
            dummy = stp.tile([128, 1], F32, name="dummy", tag="dummy")
            nc.scalar.activation(dummy[:], epsb[:], AF.Lrelu,
                                 bias=0.0, scale=1.0, alpha=SLOPE)
            nc.scalar.activation(dummy[:], epsb[:], AF.Sqrt,
                                 bias=epsb[:, 0:1])

            # BN affine params
            mean = stp.tile([128, 4], F32, name="mean", tag="mean")
            var = stp.tile([128, 4], F32, name="var", tag="var")
            sd = stp.tile([128, 4], F32, name="sd", tag="sd")
            rstd = stp.tile([128, 4], F32, name="rstd", tag="rstd")
            tmp = stp.tile([128, 4], F32, name="tmpb", tag="tmpb")
            nc.vector.tensor_scalar_mul(mean[:], ar[:, 0:4], INV_BHW)
            nc.vector.tensor_scalar_mul(var[:], ar[:, 4:8], INV_BHW)
            nc.vector.tensor_mul(tmp[:], mean[:], mean[:])
            nc.vector.tensor_sub(var[:], var[:], tmp[:])
            nc.scalar.activation(sd[:], var[:], AF.Sqrt, bias=epsb[:, 0:1])
            nc.vector.reciprocal(rstd[:], sd[:])
            nc.vector.tensor_mul(alpha[:], bngsb, rstd[:])
            nc.vector.tensor_mul(tmp[:], mean[:], alpha[:])
            nc.vector.tensor_sub(beta[:], bnbsb, tmp[:])

            if dbg is not None:
                nc.sync.dma_start(dbg["P"][:], P[:])
                nc.sync.dma_start(dbg["red"][:], red[:])
                for cc in range(NCH):
                    nc.sync.dma_start(dbg[f"X{cc}"][:], X[cc][:])
                    nc.sync.dma_start(dbg[f"ar{cc}"][:, 0:1],
                                      ar[:, cc:cc + 1])
                    nc.sync.dma_start(dbg[f"ar{cc}"][:, 1:2],
                                      ar[:, 4 + cc:5 + cc])

        # ============ phase 4: BN+leaky -> w1 -> leaky -> w2 -> y =======
        with (
            tc.tile_pool(name="y2", bufs=2) as y2p,
            tc.tile_pool(name="outb", bufs=1) as outp,
            tc.tile_pool(name="wps", bufs=1, space="PSUM") as wps,
            tc.tile_pool(name="w2ps", bufs=2, space="PSUM") as w2ps,
        ):
            osb = outp.tile([64, HW], F32, name="osb", tag="osb")

            def bn(ms):
                ssl = slice(512 * ms, 512 * (ms + 1))
                for cc in range(NCH):
                    nc.scalar.activation(X[cc][:, ssl], X[cc][:, ssl],
                                         AF.Lrelu, bias=beta[:, cc:cc + 1],
                                         scale=alpha[:, cc:cc + 1],
                                         alpha=SLOPE)

            def tail(ms):
                ssl = slice(512 * ms, 512 * (ms + 1))
                y2t = []
                for oc in range(NCH):
                    wp = wps.tile([128, 512], F32, name=f"wp{oc}",
                                  tag=f"wp{oc}")
                    for cc in range(NCH):
                        nc.tensor.matmul(
                            wp[:],
                            wpack[:, W1T0 + 512 * cc + 128 * oc:
                                  W1T0 + 512 * cc + 128 * (oc + 1)],
                            X[cc][:, ssl], start=(cc == 0), stop=(cc == 3))
                    yt = y2p.tile([128, 512], BF16, name=f"y2_{oc}",
                                  tag=f"y2_{oc}")
                    nc.scalar.activation(yt[:], wp[:], AF.Lrelu,
                                         bias=b1sb[:, oc:oc + 1],
                                         scale=1.0, alpha=SLOPE)
                    y2t.append(yt)
                fp = w2ps.tile([64, 512], F32, name="fp", tag="fp")
                for oc in range(NCH):
                    nc.tensor.matmul(fp[:],
                                     wpack[:, W2T0 + 64 * oc:W2T0 + 64 * (oc + 1)],
                                     y2t[oc][:], start=(oc == 0),
                                     stop=(oc == 3))
                nc.vector.tensor_scalar_add(osb[:, ssl], fp[:], b2sb[:, 0:1])
                nc.sync.dma_start(d["y"][:, ssl], osb[:, ssl])

            bn(0)
            bn(1)
            for ms in range(NS):
                if ms + 2 < NS:
                    bn(ms + 2)
                tail(ms)


_NC_CACHE = {}


def _build(debug=False):
    key = ("dbg" if debug else "nc")
    if key in _NC_CACHE:
        return _NC_CACHE[key]
    nc = bacc.Bacc(trn_type="TRN2", target_bir_lowering=False, debug=False,
                   enable_asserts=False, num_devices=8)
    d = {}
    d["qt"] = nc.dram_tensor("qt", (128, 2080), BF16, kind="ExternalInput").ap()
    d["kt"] = nc.dram_tensor("kt", (128, 2080), BF16, kind="ExternalInput").ap()
    d["vt"] = nc.dram_tensor("vt", (65, HW), F32R, kind="ExternalInput").ap()
    d["wpack"] = nc.dram_tensor("wpack", (128, WCOLS), BF16,
                                kind="ExternalInput").ap()
    d["spack"] = nc.dram_tensor("spack", (128, 16), F32,
                                kind="ExternalInput").ap()
    d["y"] = nc.dram_tensor("y", (64, HW), F32, kind="ExternalOutput").ap()

    dbg = None
    if debug:
        dbg = {}
        dbg["g"] = nc.dram_tensor("dbg_g", (65, 65), BF16, kind="ExternalOutput").ap()
        dbg["HTn"] = nc.dram_tensor("dbg_HTn", (65, 512), F32R, kind="ExternalOutput").ap()
        dbg["m"] = nc.dram_tensor("dbg_m", (65, 65), F32R, kind="ExternalOutput").ap()
        dbg["P"] = nc.dram_tensor("dbg_P", (65, HW), F32, kind="ExternalOutput").ap()
        dbg["red"] = nc.dram_tensor("dbg_red", (128, 8), F32, kind="ExternalOutput").ap()
        for cc in range(NCH):
            dbg[f"ET{cc}"] = nc.dram_tensor(f"dbg_ET{cc}", (128, 512), BF16, kind="ExternalOutput").ap()
            dbg[f"X{cc}"] = nc.dram_tensor(f"dbg_X{cc}", (128, HW), BF16, kind="ExternalOutput").ap()
            dbg[f"ar{cc}"] = nc.dram_tensor(f"dbg_ar{cc}", (128, 2), F32, kind="ExternalOutput").ap()
    with tile.TileContext(nc) as tc:
        _body(tc, nc, d, dbg)
    nc.compile()
    _NC_CACHE[key] = nc
    return nc


def _prep(q, k, v, wq, bq, wk, bk, wv, bv, bn_g, bn_b, w1, b1, w2, b2):
    f = np.float32
    bf = ml_dtypes.bfloat16
    wpack = np.zeros((128, WCOLS), f)
    wpack[0:65, WQE0:WQE0 + 512] = np.concatenate([wq.T, bq[None, :]], axis=0)
    wpack[0:65, WKE0:WKE0 + 512] = np.concatenate([wk.T, bk[None, :]], axis=0)
    wve = np.concatenate([wv.T, bv[None, :]], axis=0)  # [65, 512] = Wv^T aug
    for j in range(NCH):
        # wvT chunk j: [128d, 65t] = Wv~[128j:128(j+1), :] = wve.T slice
        wpack[:, WVT0 + 65 * j:WVT0 + 65 * (j + 1)] = \
            wve[:, 128 * j:128 * (j + 1)].T
    w1t = w1.T.astype(f)
    for cc in range(NCH):
        wpack[:, W1T0 + 512 * cc:W1T0 + 512 * (cc + 1)] = \
            w1t[128 * cc:128 * (cc + 1), :]
    w2t = w2.T.astype(f)
    for oc in range(NCH):
        wpack[:, W2T0 + 64 * oc:W2T0 + 64 * (oc + 1)] = \
            w2t[128 * oc:128 * (oc + 1), :]
    wpack[:, IDB0:IDB0 + 128] = np.eye(128, dtype=f)
    spack = np.zeros((128, 16), f)
    spack[:, 0:4] = b1.reshape(4, 128).T
    spack[:, 4:8] = bn_g.reshape(4, 128).T
    spack[:, 8:12] = bn_b.reshape(4, 128).T
    spack[0:64, 12] = b2

    shared = {"wpack": wpack.astype(bf), "spack": spack}
    in_maps = []
    ones_col = np.ones((HW, 1), f)
    for b in range(B):
        m = dict(shared)
        for name, x in (("qt", q[b]), ("kt", k[b])):
            xt = np.concatenate([x.reshape(C, HW).T, ones_col], axis=1)
            m[name] = np.ascontiguousarray(
                xt.reshape(NMC, 128, 65).transpose(1, 0, 2).reshape(128, 2080)
            ).astype(bf)
        m["vt"] = np.concatenate(
            [v[b].reshape(C, HW), np.ones((1, HW), f)], axis=0)
        in_maps.append(m)
    return in_maps


def _run(q, k, v, wq, bq, wk, bk, wv, bv, bn_g, bn_b, w1, b1, w2, b2,
         trace=False, tmpdir=None, debug=False):
    nc = _build(debug)
    in_maps = _prep(q, k, v, wq, bq, wk, bk, wv, bv, bn_g, bn_b, w1, b1,
                    w2, b2)
    res = bass_utils.run_bass_kernel_spmd(
        nc, in_maps, core_ids=list(range(8)), trace=trace, tmpdir=tmpdir)
    out = np.stack([res.results[b]["y"].reshape(C, 64, 64) for b in range(B)])
    return out.astype(np.float32), res


def kernel(q, k, v, wq, bq, wk, bk, wv, bv, bn_g, bn_b, w1, b1, w2, b2):
    out, _ = _run(q, k, v, wq, bq, wk, bk, wv, bv, bn_g, bn_b, w1, b1, w2, b2)
    return out
